# revision 6
# baseline (speedup 1.0000x reference)
"""DeepFM (nn_DeepFM_26027501814310) Trainium2 Bass kernel.

Contract: kernel(**inputs) takes the FULL unsharded reference inputs and
returns the FULL [16384, 1] float32 output. Batch is sharded 8 ways across
NeuronCores (data-parallel; the embedding table is replicated); a Bass/Tile
kernel computes the full DeepFM forward per shard; shard outputs concat.

Per-core layout (SH=2048 rows, chunks of CH=512 along the free dim):
- One augmented table aug[f*V+v] = [v_cat row (64) | lin_cat | -0.5*||row||^2]
  (66 f32r) built on host. Indices are host-flattened (f*V + x_cat).
- Gathers use the hardware-proven one-index-per-partition indirect DMA:
  one op per (sub-group of 128 batch rows, field) -> [128, 66] tile.
  (Multi-index offset APs are mis-walked by the TRN2 SWDGE ucode.)
- Each field tile is transposed to feature-major via a normal f32r matmul
  against an identity (M=64, N=128); even/odd fields land in PSUM partition
  halves so pairs share one PSUM->SBUF copy into vcT [128, CH] chunks.
- 3-layer MLP runs feature-major in f32r (moving dim 512); relu+bias on ACT
  from PSUM. FM interaction and first-order terms all fold into one final
  [1, CH] PSUM accumulation group on the PE.
"""
import numpy as np
import concourse.bass as bass
import concourse.bacc as bacc
import concourse.mybir as mybir
import concourse.tile as tile
from concourse.bass_utils import run_bass_kernel_spmd

F32R = mybir.dt.float32r
F32 = mybir.dt.float32
I32 = mybir.dt.int32
AF = mybir.ActivationFunctionType
P = 128

B = 16384
N_CORES = 8
NF = 26
K = 64
NNUM = 13
VOC = 100000
HID = (1024, 512, 256)
SH = B // N_CORES
CH = 512
AW = K + 2  # augmented row width


def build_kernel(SH=SH, CH=CH, NF=NF, K=K, NNUM=NNUM, HID=HID, VOC=VOC):
    NSUB = CH // P
    NCH = SH // CH
    DCAT = NF * K
    NKC = DCAT // P
    assert DCAT % P == 0 and CH % P == 0 and SH % CH == 0

    nc = bacc.Bacc("TRN2", target_bir_lowering=False, debug=False, num_devices=8,
                   dynamic_dma_scratch_size=49152)

    ta = nc.dram_tensor("table_aug", [NF * VOC, AW], F32R, kind="ExternalInput").ap()
    idx_d = nc.dram_tensor("idx", [NCH, P, NSUB * NF], I32, kind="ExternalInput").ap()
    xnt_d = nc.dram_tensor("xnumT", [NNUM, SH], F32R, kind="ExternalInput").ap()
    w1n_d = nc.dram_tensor("W1num", [NNUM, HID[0]], F32R, kind="ExternalInput").ap()
    w1c_d = nc.dram_tensor("W1cat", [DCAT, HID[0]], F32R, kind="ExternalInput").ap()
    w2_d = nc.dram_tensor("W2", [HID[0], HID[1]], F32R, kind="ExternalInput").ap()
    w3_d = nc.dram_tensor("W3", [HID[1], HID[2]], F32R, kind="ExternalInput").ap()
    wo_d = nc.dram_tensor("Wo", [HID[2], 1], F32R, kind="ExternalInput").ap()
    b1_d = nc.dram_tensor("b1t", [P, HID[0] // P], F32, kind="ExternalInput").ap()
    b2_d = nc.dram_tensor("b2t", [P, HID[1] // P], F32, kind="ExternalInput").ap()
    b3_d = nc.dram_tensor("b3t", [P, HID[2] // P], F32, kind="ExternalInput").ap()
    bs_d = nc.dram_tensor("bias_sum", [1, 1], F32, kind="ExternalInput").ap()
    vnum_d = nc.dram_tensor("v_num", [NNUM, K], F32R, kind="ExternalInput").ap()
    wnum_d = nc.dram_tensor("w_num", [NNUM, 1], F32R, kind="ExternalInput").ap()
    nsq_d = nc.dram_tensor("nsq_num", [NNUM, 1], F32R, kind="ExternalInput").ap()
    ident_d = nc.dram_tensor("ident", [P, P], F32R, kind="ExternalInput").ap()
    sel_d = nc.dram_tensor("sel", [P, K], F32R, kind="ExternalInput").ap()
    ho_d = nc.dram_tensor("half_ones", [K, 1], F32R, kind="ExternalInput").ap()
    ones2_d = nc.dram_tensor("ones2", [2, 1], F32R, kind="ExternalInput").ap()
    out_d = nc.dram_tensor("out", [NCH, CH], F32, kind="ExternalOutput").ap()

    NM1, NM2, NM3 = HID[0] // P, HID[1] // P, HID[2] // P
    NK2, NK3 = HID[0] // P, HID[1] // P

    with tile.TileContext(nc) as tc:
        with tc.tile_pool(name="wpool", bufs=1) as wp, \
             tc.tile_pool(name="gpool", bufs=NF + 8) as gp, \
             tc.tile_pool(name="idxpool", bufs=2) as ixp, \
             tc.tile_pool(name="xtpool", bufs=1) as xtp, \
             tc.tile_pool(name="hpool", bufs=1) as hp, \
             tc.tile_pool(name="smallpool", bufs=1) as smp, \
             tc.tile_pool(name="opool", bufs=2) as op, \
             tc.tile_pool(name="voddpool", bufs=4) as vop, \
             tc.tile_pool(name="tp_ps", bufs=3, space="PSUM") as tpp, \
             tc.tile_pool(name="mlp_ps", bufs=2, space="PSUM") as mpp, \
             tc.tile_pool(name="fm_ps", bufs=1, space="PSUM") as fmp:

            w1c = []
            for c in range(NKC):
                t = wp.tile([P, HID[0]], F32R, tag=f"w1c{c}")
                nc.sync.dma_start(out=t[:], in_=w1c_d[c * P:(c + 1) * P, :])
                w1c.append(t)
            w1n = wp.tile([NNUM, HID[0]], F32R, tag="w1n")
            nc.sync.dma_start(out=w1n[:], in_=w1n_d[:])
            w2 = []
            for c in range(NK2):
                t = wp.tile([P, HID[1]], F32R, tag=f"w2{c}")
                nc.sync.dma_start(out=t[:], in_=w2_d[c * P:(c + 1) * P, :])
                w2.append(t)
            w3 = []
            for c in range(NK3):
                t = wp.tile([P, HID[2]], F32R, tag=f"w3{c}")
                nc.sync.dma_start(out=t[:], in_=w3_d[c * P:(c + 1) * P, :])
                w3.append(t)
            wo = []
            for c in range(HID[2] // P):
                t = wp.tile([P, 1], F32R, tag=f"wo{c}")
                nc.sync.dma_start(out=t[:], in_=wo_d[c * P:(c + 1) * P, :])
                wo.append(t)
            b1 = wp.tile([P, NM1], F32, tag="b1")
            nc.sync.dma_start(out=b1[:], in_=b1_d[:])
            b2 = wp.tile([P, NM2], F32, tag="b2")
            nc.sync.dma_start(out=b2[:], in_=b2_d[:])
            b3 = wp.tile([P, NM3], F32, tag="b3")
            nc.sync.dma_start(out=b3[:], in_=b3_d[:])
            bs = wp.tile([1, 1], F32, tag="bs")
            nc.sync.dma_start(out=bs[:], in_=bs_d[:])
            vnum = wp.tile([NNUM, K], F32R, tag="vnum")
            nc.sync.dma_start(out=vnum[:], in_=vnum_d[:])
            wnum = wp.tile([NNUM, 1], F32R, tag="wnum")
            nc.sync.dma_start(out=wnum[:], in_=wnum_d[:])
            nsq = wp.tile([NNUM, 1], F32R, tag="nsq")
            nc.sync.dma_start(out=nsq[:], in_=nsq_d[:])
            ident = wp.tile([P, P], F32R, tag="ident")
            nc.sync.dma_start(out=ident[:], in_=ident_d[:])
            sel = wp.tile([P, K], F32R, tag="sel")
            nc.sync.dma_start(out=sel[:], in_=sel_d[:])
            ho = wp.tile([K, 1], F32R, tag="ho")
            nc.sync.dma_start(out=ho[:], in_=ho_d[:])
            ones2 = wp.tile([2, 1], F32R, tag="ones2")
            nc.sync.dma_start(out=ones2[:], in_=ones2_d[:])

            xnt = xtp.tile([NNUM, SH], F32R, tag="xnt")
            nc.sync.dma_start(out=xnt[:], in_=xnt_d[:])

            for j in range(NCH):
                idx_t = ixp.tile([P, NSUB * NF], I32, tag="idx")
                nc.sync.dma_start(out=idx_t[:], in_=idx_d[j])

                vct = []
                for c in range(NKC):
                    t = xtp.tile([P, CH], F32R, tag=f"vct{c}")
                    vct.append(t)

                linsq_ps = fmp.tile([2, CH], F32, tag="linsq_ps")
                for s in range(NSUB):
                    # one [128, AW] gather per field (proven 1-idx/partition)
                    gfs = []
                    for f in range(NF):
                        gf = gp.tile([P, AW], F32R, tag="gf")
                        nc.gpsimd.indirect_dma_start(
                            out=gf[:], out_offset=None, in_=ta[:],
                            in_offset=bass.IndirectOffsetOnAxis(
                                ap=idx_t[:, s * NF + f:s * NF + f + 1], axis=0))
                        gfs.append(gf)

                    # transposes (M=64, base partition 0). Even fields copy
                    # straight into vct rows 0:64; odd fields bounce through
                    # an SBUF scratch and DMA-shift to partitions 64:128
                    # (DVE/ACT cannot move data across partitions; matmul
                    # dst base 64 fails the 4-byte ISA check).
                    for c in range(NKC):
                        tp = tpp.tile([K, 2 * P], F32, tag="tp")
                        nc.tensor.matmul(tp[:, 0:P], gfs[2 * c][:, 0:K],
                                         ident[:], start=True, stop=True)
                        nc.tensor.matmul(tp[:, P:2 * P], gfs[2 * c + 1][:, 0:K],
                                         ident[:], start=True, stop=True)
                        if c % 2 == 0:
                            nc.vector.tensor_copy(
                                vct[c][0:K, s * P:(s + 1) * P], tp[:, 0:P])
                        else:
                            nc.scalar.copy(
                                vct[c][0:K, s * P:(s + 1) * P], tp[:, 0:P])
                        vodd = vop.tile([K, P], F32R, tag="vodd")
                        if c % 2 == 0:
                            nc.scalar.copy(vodd[:], tp[:, P:2 * P])
                        else:
                            nc.vector.tensor_copy(vodd[:], tp[:, P:2 * P])
                        nc.scalar.dma_start(
                            out=vct[c][K:P, s * P:(s + 1) * P], in_=vodd[:])

                    # lin / -0.5*sqnorm: pairwise-add tree over the 26 [128,2]
                    # tails, then a tiny transpose into linsq_ps columns
                    acc_ls = smp.tile([P, 2], F32, tag="acc_ls")
                    nc.vector.tensor_tensor(
                        out=acc_ls[:], in0=gfs[0][:, K:AW].bitcast(F32),
                        in1=gfs[1][:, K:AW].bitcast(F32), op=mybir.AluOpType.add)
                    for f in range(2, NF):
                        nc.vector.tensor_tensor(
                            out=acc_ls[:], in0=acc_ls[:],
                            in1=gfs[f][:, K:AW].bitcast(F32),
                            op=mybir.AluOpType.add)
                    redr = smp.tile([P, 2], F32R, tag="redr")
                    nc.vector.tensor_copy(redr[:], acc_ls[:])
                    nc.tensor.matmul(linsq_ps[:, s * P:(s + 1) * P], redr[:],
                                     ident[:], start=True, stop=True)

                linsq = smp.tile([2, CH], F32R, tag="linsq")
                nc.vector.tensor_copy(linsq[:], linsq_ps[:])

                xn = xnt[:, j * CH:(j + 1) * CH]
                xn2 = smp.tile([NNUM, CH], F32R, tag="xn2")
                nc.vector.tensor_tensor(out=xn2[:], in0=xn.bitcast(F32),
                                        in1=xn.bitcast(F32),
                                        op=mybir.AluOpType.mult)

                sv_ps = fmp.tile([K, CH], F32, tag="sv_ps")
                nc.tensor.matmul(sv_ps[:], vnum[:], xn, start=True, stop=False)
                for c in range(NKC):
                    nc.tensor.matmul(sv_ps[:], sel[:], vct[c][:],
                                     start=False, stop=(c == NKC - 1))
                sv2 = smp.tile([K, CH], F32R, tag="sv2")
                nc.scalar.activation(sv2[:], sv_ps[:], AF.Square)

                h1 = []
                for m in range(NM1):
                    ps = mpp.tile([P, CH], F32, tag="mlp_ps")
                    nc.tensor.matmul(ps[:], w1n[:, m * P:(m + 1) * P], xn,
                                     start=True, stop=False)
                    for c in range(NKC):
                        nc.tensor.matmul(ps[:], w1c[c][:, m * P:(m + 1) * P],
                                         vct[c][:], start=False, stop=(c == NKC - 1))
                    h = hp.tile([P, CH], F32R, tag=f"h1_{m}")
                    nc.scalar.activation(h[:], ps[:], AF.Relu, bias=b1[:, m:m + 1])
                    h1.append(h)
                h2 = []
                for m in range(NM2):
                    ps = mpp.tile([P, CH], F32, tag="mlp_ps")
                    for c in range(NK2):
                        nc.tensor.matmul(ps[:], w2[c][:, m * P:(m + 1) * P],
                                         h1[c][:], start=(c == 0), stop=(c == NK2 - 1))
                    h = hp.tile([P, CH], F32R, tag=f"h2_{m}")
                    nc.scalar.activation(h[:], ps[:], AF.Relu, bias=b2[:, m:m + 1])
                    h2.append(h)
                h3 = []
                for m in range(NM3):
                    ps = mpp.tile([P, CH], F32, tag="mlp_ps")
                    for c in range(NK3):
                        nc.tensor.matmul(ps[:], w3[c][:, m * P:(m + 1) * P],
                                         h2[c][:], start=(c == 0), stop=(c == NK3 - 1))
                    h = hp.tile([P, CH], F32R, tag=f"h3_{m}")
                    nc.scalar.activation(h[:], ps[:], AF.Relu, bias=b3[:, m:m + 1])
                    h3.append(h)

                acc = fmp.tile([1, CH], F32, tag="acc")
                nc.tensor.matmul(acc[:], wo[0][:], h3[0][:], start=True, stop=False)
                for c in range(1, HID[2] // P):
                    nc.tensor.matmul(acc[:], wo[c][:], h3[c][:],
                                     start=False, stop=False)
                nc.tensor.matmul(acc[:], wnum[:], xn, start=False, stop=False)
                nc.tensor.matmul(acc[:], nsq[:], xn2[:], start=False, stop=False)
                nc.tensor.matmul(acc[:], ho[:], sv2[:], start=False, stop=False)
                nc.tensor.matmul(acc[:], ones2[:], linsq[:], start=False, stop=True)

                ot = op.tile([1, CH], F32, tag="ot")
                nc.scalar.activation(ot[:], acc[:], AF.Identity, bias=bs[:])
                nc.sync.dma_start(out=out_d[j:j + 1, :], in_=ot[:])

    nc.compile()
    return nc


def make_host_inputs(inputs, n_cores=N_CORES):
    x_num = np.asarray(inputs["x_num"], np.float32)
    x_cat = np.asarray(inputs["x_cat"], np.int32)
    v_cat = np.asarray(inputs["v_cat"], np.float32)
    v_num = np.asarray(inputs["v_num"], np.float32)
    W1 = np.asarray(inputs["W1"], np.float32)
    B_, NNUM_ = x_num.shape
    NF_ = x_cat.shape[1]
    VOC_ = v_cat.shape[1]
    K_ = v_cat.shape[2]
    H1, H2, H3 = (np.asarray(inputs["W1"]).shape[1],
                  np.asarray(inputs["W2"]).shape[1],
                  np.asarray(inputs["W3"]).shape[1])
    SH_ = B_ // n_cores
    NCH_ = SH_ // CH
    NSUB_ = CH // P

    table_v = v_cat.reshape(NF_ * VOC_, K_)
    lin_cat = np.asarray(inputs["lin_cat"], np.float32).reshape(NF_ * VOC_)
    sqn = -0.5 * (table_v.astype(np.float64) ** 2).sum(axis=1)
    aug = np.empty((NF_ * VOC_, K_ + 2), np.float32)
    aug[:, :K_] = table_v
    aug[:, K_] = lin_cat
    aug[:, K_ + 1] = sqn.astype(np.float32)

    fofs = (np.arange(NF_, dtype=np.int64) * VOC_)[None, :]
    flat = (x_cat.astype(np.int64) + fofs).astype(np.int32)

    shared = {
        "table_aug": aug,
        "W1num": np.ascontiguousarray(W1[:NNUM_]),
        "W1cat": np.ascontiguousarray(W1[NNUM_:]),
        "W2": np.asarray(inputs["W2"], np.float32),
        "W3": np.asarray(inputs["W3"], np.float32),
        "Wo": np.asarray(inputs["Wo"], np.float32),
        "b1t": np.ascontiguousarray(
            np.asarray(inputs["b1"], np.float32).reshape(H1 // P, P).T),
        "b2t": np.ascontiguousarray(
            np.asarray(inputs["b2"], np.float32).reshape(H2 // P, P).T),
        "b3t": np.ascontiguousarray(
            np.asarray(inputs["b3"], np.float32).reshape(H3 // P, P).T),
        "bias_sum": np.array(
            [[float(np.asarray(inputs["bias"]).reshape(-1)[0]) +
              float(np.asarray(inputs["bo"]).reshape(-1)[0])]], np.float32),
        "v_num": v_num,
        "w_num": np.asarray(inputs["w_num"], np.float32),
        "nsq_num": (-0.5 * (v_num.astype(np.float64) ** 2).sum(
            axis=1, keepdims=True)).astype(np.float32),
        "ident": np.eye(P, dtype=np.float32),
        "sel": np.concatenate([np.eye(K_, dtype=np.float32)] * (P // K_), axis=0),
        "half_ones": np.full((K_, 1), 0.5, np.float32),
        "ones2": np.ones((2, 1), np.float32),
    }

    in_maps = []
    for c in range(n_cores):
        sl = slice(c * SH_, (c + 1) * SH_)
        fl = flat[sl].reshape(NCH_, NSUB_, P, NF_)
        idx = np.ascontiguousarray(
            fl.transpose(0, 2, 1, 3).reshape(NCH_, P, NSUB_ * NF_))
        m = dict(shared)
        m["idx"] = idx
        m["xnumT"] = np.ascontiguousarray(x_num[sl].T)
        in_maps.append(m)
    return in_maps


_CACHED_NC = None


def _get_nc():
    global _CACHED_NC
    if _CACHED_NC is None:
        _CACHED_NC = build_kernel()
    return _CACHED_NC


def run_sharded(inputs, trace=False, **kwargs):
    nc = _get_nc()
    in_maps = make_host_inputs(inputs)
    res = run_bass_kernel_spmd(nc, in_maps, core_ids=list(range(N_CORES)),
                               trace=trace, **kwargs)
    outs = [res.results[c]["out"].reshape(SH, 1) for c in range(N_CORES)]
    return np.concatenate(outs, axis=0).astype(np.float32), res


def kernel(**inputs):
    out, _ = run_sharded(inputs, trace=False)
    return out
